# revision 1
# baseline (speedup 1.0000x reference)
"""Trainium2 Bass kernel for nn_DeChunkLayer (segment-reset linear scan + dechunk gather).

Math (from the reference):
    p  = clip(p_selected, EPS, 1-EPS);  dt = -log1p(-p)
    y_t = a_t * y_{t-1} + b_t  with  a_t = exp(-dt_t) (0 at segment starts),
                                     b_t = (dt_t*p_t) * (h_t/dt_t)  (~= p_t*h_t)
    out[j] = y[cumsum(b_flat)[j]-1]    (negative -> wraps; each row ~duplicated)

Device strategy (8 NeuronCores, sequence-parallel at segment boundaries):
  - Each core gets a contiguous token range starting at a segment boundary
    (fresh scan state), padded to a fixed number of 127-token chunks.
  - Per chunk, the scan is a matmul:  y[t] = sum_s M[s,t] * B[s]  where
    s=0 is a carry pseudo-row holding the previous chunk's last state and
    s=1..127 are the chunk's tokens.  M is built on-device from host-derived
    per-token values (chunk-local decay cumsum, global reset count, ln p):
        M[s,t] = exp(min(cum_t - cum_s, 0) + lnp_s) * (R_t == R_s + BIG*(s>t+1))
    The BIG term poisons non-causal entries so one is_equal builds the
    combined causal+segment mask.  The chunk chain is serial only through
    one [1,512] copy of the state row into the next chunk's rhs.
  - y is stored once per core in a partition-major layout (batched, few
    large DMAs -- the HW store path is the bottleneck at ~20 GB/s effective),
    and the dechunk duplication/gather out[j] = y[idx[j]] happens in the host
    unshard step, halving device write traffic.
"""

import math

import numpy as np

import concourse.bass as bass
import concourse.tile as tile
from concourse import mybir
from concourse.bass_utils import run_bass_kernel_spmd

EPS = 1e-4
N_CORES = 8
D = 512
C = 127          # real tokens per chunk (matrix row s=0 is the carry row)
BATCH = 12       # chunks per DMA batch (DGE descriptor generation is the
                 # real-HW bottleneck: ~10us per dma_start; batch aggressively)
BIG = 65536.0

F32 = mybir.dt.float32

_prog_cache: dict = {}
last_results = None  # BassKernelResults of the most recent device run (for test harness)


def _legalize_waits(nc: bass.Bass) -> None:
    """walrus codegen allows one sync-wait per engine instruction; move any
    surplus waits onto injected same-engine no-ops right before it."""
    nid = 0
    for fn in nc.m.functions:
        for blk in fn.blocks:
            out = []
            changed = False
            for inst in blk.instructions:
                si = getattr(inst, "sync_info", None)
                waits = list(si.on_wait) if si is not None and si.on_wait else []
                if len(waits) > 1:
                    for w in waits[:-1]:
                        nop = mybir.InstNoOp(
                            name=f"waitnop-{nid}", text_hint="waitsplit"
                        )
                        nid += 1
                        nop.engine = inst.engine
                        nop.sync_info = mybir.SyncInfo(on_wait=[w], on_update=[])
                        out.append(nop)
                    inst.sync_info = mybir.SyncInfo(
                        on_wait=[waits[-1]], on_update=list(si.on_update)
                    )
                    changed = True
                out.append(inst)
            if changed:
                blk.instructions = out


def _build_program(nchunk: int, dup: bool, legalize: bool = True, loop_n: int = 0) -> bass.Bass:
    t_pad = nchunk * C
    ow = 2 * D if dup else D
    nbatch = nchunk // BATCH
    assert nchunk % BATCH == 0

    q4 = (nchunk + 2) // 3
    nc = bass.Bass("TRN2", target_bir_lowering=False, debug=False, num_devices=N_CORES)
    h_dev = nc.dram_tensor("h_dev", [C, nchunk * D], F32, kind="ExternalInput")
    # per-chunk rows packed 3-way across partitions {0,32,64} (matmul operand
    # base-partition rule; quadrant 3 unsupported) to keep SBUF columns low
    cumr4 = nc.dram_tensor("cumr4", [128, q4 * C], F32, kind="ExternalInput")
    rr4 = nc.dram_tensor("rr4", [128, q4 * C], F32, kind="ExternalInput")
    colv = nc.dram_tensor("colv", [128, nchunk * 3], F32, kind="ExternalInput")
    ones1 = nc.dram_tensor("ones1", [128, 128], F32, kind="ExternalInput")
    killa = nc.dram_tensor("killa", [128, 128], F32, kind="ExternalInput")
    bigi = nc.dram_tensor("bigi", [128, C], F32, kind="ExternalInput")
    # partition-major: out[k, c*ow:...] = output row of token (k+30)%127 of
    # chunk c; the host un-rotates and transposes during final assembly
    out = nc.dram_tensor("out", [C, nchunk * ow], F32, kind="ExternalOutput")

    with tile.TileContext(nc) as tc:
        with (
            tc.tile_pool(name="consts", bufs=1) as consts,
            tc.tile_pool(name="hpool", bufs=2) as hpool,
            tc.tile_pool(name="mpool", bufs=3) as mpool,
            tc.tile_pool(name="ypool", bufs=2) as ypool,
            tc.tile_pool(name="px", bufs=3, space="PSUM") as px,
            tc.tile_pool(name="py", bufs=3, space="PSUM") as py,
        ):
            ones_sb = consts.tile([128, 128], F32)
            nc.sync.dma_start(ones_sb, ones1[:, :])
            killa_sb = consts.tile([128, 128], F32)
            nc.sync.dma_start(killa_sb, killa[:, :])
            bigi_sb = consts.tile([128, C], F32)
            nc.sync.dma_start(bigi_sb, bigi[:, :])
            cumr_sb = consts.tile([128, q4 * C], F32)
            nc.sync.dma_start(cumr_sb, cumr4[:, :])
            rr_sb = consts.tile([128, q4 * C], F32)
            nc.sync.dma_start(rr_sb, rr4[:, :])
            colv_sb = consts.tile([128, nchunk * 3], F32)
            nc.sync.dma_start(colv_sb, colv[:, :])

            def load_batch(b):
                t = hpool.tile([128, BATCH * D], F32, tag="rhs")
                nc.sync.dma_start(
                    t[1:128, :], h_dev[:, b * BATCH * D : (b + 1) * BATCH * D]
                )
                if b == 0:
                    nc.vector.memset(t[0:1, 0:D], 0.0)
                return t

            import contextlib

            loop_ctx = tc.For_i(0, loop_n, 1) if loop_n else contextlib.nullcontext()
            with loop_ctx:
              rhs = load_batch(0)
              for b in range(nbatch):
                nxt = load_batch(b + 1) if b + 1 < nbatch else None
                y2 = ypool.tile([C, BATCH * ow], F32, tag="y2")
                for ci in range(BATCH):
                    c = b * BATCH + ci
                    # X1[s,t] = cum_row[t];  X2[s,t] = R_row[t] + BIG*(s>t+1)
                    # (both halves of one PSUM bank)
                    x12 = px.tile([128, 256], F32, tag="x12")
                    x1 = x12[:, 0:C]
                    x2 = x12[:, C : 2 * C]
                    pr = 32 * (c % 3)
                    qc = c // 3
                    ones_blk = ones_sb[pr : pr + 1, :]
                    nc.tensor.matmul(
                        x1, ones_blk, cumr_sb[pr : pr + 1, qc * C : (qc + 1) * C],
                        start=True, stop=True,
                    )
                    nc.tensor.matmul(
                        x2, ones_blk, rr_sb[pr : pr + 1, qc * C : (qc + 1) * C],
                        start=True, stop=False,
                    )
                    nc.tensor.matmul(x2, killa_sb, bigi_sb, start=False, stop=True)
                    # D = min(cum_row - cum_col, 0);  E = exp(D + lnp_col)
                    dmat = mpool.tile([128, C], F32, tag="d")
                    nc.vector.tensor_scalar(
                        dmat, x1, colv_sb[:, 3 * c : 3 * c + 1], 0.0,
                        mybir.AluOpType.subtract, mybir.AluOpType.min,
                    )
                    emat = mpool.tile([128, C], F32, tag="e")
                    nc.scalar.activation(
                        emat, dmat, mybir.ActivationFunctionType.Exp,
                        bias=colv_sb[:, 3 * c + 2 : 3 * c + 3], scale=1.0,
                    )
                    # mask = (X2 == R_col);  M = E * mask
                    mmat = mpool.tile([128, C], F32, tag="m")
                    nc.vector.tensor_scalar(
                        mmat, x2, colv_sb[:, 3 * c + 1 : 3 * c + 2], None,
                        mybir.AluOpType.is_equal,
                    )
                    lmat = mpool.tile([128, C], F32, tag="l")
                    nc.vector.tensor_tensor(lmat, emat, mmat, mybir.AluOpType.mult)
                    # y[t,:] = sum_s M[s,t] * rhs[s,:].  Matmul column k holds
                    # token (k+30)%127, so the state row (token 126) lands at
                    # partition 96 -- a legal engine-copy base (0/32/64/96).
                    rhs_blk = rhs[:, ci * D : (ci + 1) * D]
                    yp = py.tile([C, D], F32, tag="y")
                    nc.tensor.matmul(yp, lmat, rhs_blk, start=True, stop=True)
                    if ci + 1 < BATCH:
                        nc.vector.tensor_copy(
                            rhs[0:1, (ci + 1) * D : (ci + 2) * D], yp[96:97, :]
                        )
                    elif nxt is not None:
                        nc.vector.tensor_copy(nxt[0:1, 0:D], yp[96:97, :])
                    # emit output rows ([y|y] when dup) into the batch tile
                    nc.scalar.copy(y2[:, ci * ow : ci * ow + D], yp)
                    if dup:
                        nc.scalar.copy(y2[:, ci * ow + D : (ci + 1) * ow], yp)
                # alternate store batches across the two HWDGE rings
                # (SP/ACT): measured 2.35ms -> 1.76ms per pass on HW
                (nc.sync if b % 2 == 0 else nc.scalar).dma_start(
                    out[:, b * BATCH * ow : (b + 1) * BATCH * ow], y2
                )
                if nxt is not None:
                    rhs = nxt
    if legalize:
        _legalize_waits(nc)
    return nc


def _get_program(nchunk: int, dup: bool) -> bass.Bass:
    key = (nchunk, dup)
    if key not in _prog_cache:
        _prog_cache[key] = _build_program(nchunk, dup)
    return _prog_cache[key]


def _split_ranges(starts: np.ndarray, length: int, k: int):
    """Partition [0,length) into k contiguous ranges cutting only at segment
    starts, minimizing the max range length. Returns list of (t0, t1)."""
    bounds = np.append(starts, length)
    lens = np.diff(bounds)
    nseg = len(lens)
    if nseg <= k:
        ranges = [(int(bounds[i]), int(bounds[i + 1])) for i in range(nseg)]
        ranges += [(length, length)] * (k - nseg)
        return ranges
    lo, hi = int(lens.max()), int(length)
    while lo < hi:
        mid = (lo + hi) // 2
        groups, cur = 1, 0
        for ln in lens:
            if cur + ln <= mid:
                cur += ln
            else:
                groups += 1
                cur = ln
        if groups <= k:
            hi = mid
        else:
            lo = mid + 1
    ranges = []
    s, cur = int(bounds[0]), 0
    for i, ln in enumerate(lens):
        if cur + ln > lo:
            ranges.append((s, int(bounds[i])))
            s, cur = int(bounds[i]), 0
        cur += int(ln)
    ranges.append((s, length))
    ranges += [(length, length)] * (k - len(ranges))
    return ranges


def _core_inputs(h_flat, dt64, Rg, lnp, t0, t1, nchunk):
    t_pad = nchunk * C
    n = t1 - t0

    dtl = np.zeros(t_pad, np.float64)
    dtl[:n] = dt64[t0:t1]
    Rl = np.full(t_pad, -2.0, np.float64)
    Rl[:n] = Rg[t0:t1]
    lnl = np.zeros(t_pad, np.float64)
    lnl[:n] = lnp[t0:t1]

    cum = -np.cumsum(dtl.reshape(nchunk, C), axis=1)  # chunk-local decay logsum
    mc = cum.mean(axis=1, keepdims=True)              # center for f32 precision
    perm = (np.arange(C) + 30) % C                    # matmul column k <-> token perm[k]
    rowcum = (cum - mc)[:, perm]
    rowR = Rl.reshape(nchunk, C)[:, perm]
    # pack chunk c's row vectors at partition 32*(c%3), column block c//3
    q4 = (nchunk + 2) // 3
    cumr4 = np.zeros((128, q4 * C), np.float32)
    rr4 = np.zeros((128, q4 * C), np.float32)
    cidx = np.arange(nchunk)
    for r in range(3):
        sel = cidx[cidx % 3 == r]
        qs = sel // 3
        cumr4[32 * r].reshape(q4, C)[qs] = rowcum[sel]
        rr4[32 * r].reshape(q4, C)[qs] = rowR[sel]

    # matrix row s>=1 of chunk c sources local token c*C+(s-1); its cum-col
    # value is the chunk-local cumsum at that token. s=0 is the carry row.
    colv = np.zeros((128, nchunk, 3), np.float64)
    colv[0, :, 0] = -mc[:, 0]
    colv[1:, :, 0] = (cum - mc).T
    rprev = np.empty(nchunk, np.float64)
    rprev[0] = -1.0                      # kill carry into the first chunk
    rprev[1:] = Rl.reshape(nchunk, C)[:-1, -1]
    colv[0, :, 1] = rprev
    colv[1:, :, 1] = Rl.reshape(nchunk, C).T
    colv[0, :, 2] = 0.0
    colv[1:, :, 2] = lnl.reshape(nchunk, C).T
    colv = colv.reshape(128, nchunk * 3).astype(np.float32)

    hl = np.zeros((t_pad, D), np.float32)
    hl[:n] = h_flat[t0:t1]
    h_dev = np.ascontiguousarray(
        hl.reshape(nchunk, C, D).transpose(1, 0, 2)
    ).reshape(C, nchunk * D)
    return h_dev, cumr4, rr4, colv


def kernel(h_flat, b_flat, p_selected_flat, h_seq_idx):
    global last_results
    h_flat = np.ascontiguousarray(h_flat, np.float32)
    L, d = h_flat.shape
    assert d == D
    seg = np.asarray(h_seq_idx).reshape(-1).astype(np.int64)

    lo_f = np.float32(EPS)
    hi_f = np.float32(1.0 - EPS)
    p64 = np.clip(np.asarray(p_selected_flat, np.float32), lo_f, hi_f).astype(np.float64)
    dt64 = -np.log1p(-p64)
    lnp = np.log(p64)

    startf = np.empty(L, bool)
    startf[0] = True
    startf[1:] = seg[1:] != seg[:-1]
    Rg = np.cumsum(startf).astype(np.float64)

    idx = np.cumsum(np.asarray(b_flat, np.int64)) - 1
    Lo = idx.shape[0]
    # The HW store path runs at ~20 GB/s (write-side platform limit), so the
    # 2x output duplication is done in the host gather instead of on-device:
    # the device writes y once (34 MB/core) rather than the 67 MB dup form.
    dup = False

    ranges = _split_ranges(np.flatnonzero(startf), L, N_CORES)
    maxlen = max(t1 - t0 for t0, t1 in ranges)
    nchunk = max(((math.ceil(maxlen / C) + BATCH - 1) // BATCH) * BATCH, BATCH)
    t_pad = nchunk * C

    nc = _get_program(nchunk, dup)

    ones1 = np.ones((128, 128), np.float32)
    killa = (
        np.arange(128)[:, None] < (np.arange(128)[None, :] - 1)
    ).astype(np.float32)
    perm = (np.arange(C) + 30) % C
    bigi = (BIG * np.eye(128, C)[:, perm]).astype(np.float32)

    in_maps = []
    for t0, t1 in ranges:
        h_dev, cumr4, rr4, colv = _core_inputs(h_flat, dt64, Rg, lnp, t0, t1, nchunk)
        in_maps.append(
            {
                "h_dev": h_dev,
                "cumr4": cumr4,
                "rr4": rr4,
                "colv": colv,
                "ones1": ones1,
                "killa": killa,
                "bigi": bigi,
            }
        )

    import os

    trace = bool(os.environ.get("BASSK_TRACE"))
    try:
        res = run_bass_kernel_spmd(
            nc, in_maps, core_ids=list(range(N_CORES)), trace=trace
        )
    except ModuleNotFoundError:
        res = run_bass_kernel_spmd(
            nc, in_maps, core_ids=list(range(N_CORES)), trace=False
        )
    last_results = res

    ow = 2 * D if dup else D

    def natural(dev):
        # dev [C, nchunk*ow]: partition k, chunk c = token (k+30)%C of chunk c
        dev3 = np.roll(dev.reshape(C, nchunk, ow), 30, axis=0)
        return dev3.transpose(1, 0, 2)  # [nchunk, C, ow] view

    if dup:
        final = np.empty((Lo, D), np.float32)
        for i, (t0, t1) in enumerate(ranges):
            n = t1 - t0
            if n:
                final[2 * t0 : 2 * t1] = natural(res.results[i]["out"]).reshape(
                    2 * t_pad, D
                )[: 2 * n]
        return final
    y = np.empty((L, D), np.float32)
    for i, (t0, t1) in enumerate(ranges):
        n = t1 - t0
        if n:
            y[t0:t1] = natural(res.results[i]["out"]).reshape(t_pad, D)[:n]
    gidx = np.where(idx < 0, idx + L, idx)
    gidx = np.clip(gidx, 0, L - 1)
    return y[gidx]



# revision 2
# speedup vs baseline: 3.7151x; 3.7151x over previous
"""Trainium2 Bass kernel for nn_DeChunkLayer — DVE linear-scan version.

Math (from the reference):
    p  = clip(p_selected, EPS, 1-EPS)
    y_t = a_t * y_{t-1} + b_t,  a_t = 1-p_t (0 at segment starts),
                                b_t = p_t * h_t
    out[j] = y[cumsum(b_flat)[j]-1]   (host-side gather; rows ~duplicated)

Device strategy (8 NeuronCores, sequence-parallel at segment boundaries):
  - Each core gets a contiguous token range starting at a segment boundary
    (the scan state resets there, so ranges are independent).
  - Tokens live on the FREE dim, d-channels on partitions (4 groups of 128).
  - The scan itself is the DVE `tensor_tensor_scan` instruction:
        state = (a_t * state) + b_t    per partition, fp32 internal state
    chained across 512-token links via `initial=prev[:, -1:]`.
  - b is computed on device as (a-1)*h = -p*h in one scalar_tensor_tensor
    op (the scan then tracks -y; the host negates while unpacking). At the
    few segment-start rows a is forced to 0, so the host folds the true
    gate p into h there.
  - a is uploaded pre-broadcast [128, T] fp16; h/y move as fp16 (the
    correctness gate is 2e-2 relative; fp16 end-to-end lands ~4e-4).
  - Streams: h+a loads on the SP HWDGE ring, y stores on the ACT ring,
    slabs of 8 links double-buffered; ~39 MB/core round trip.

Measured on HW (per-pass, 8 cores, For_i loop-delta timing): ~240-265 us
vs ~2.26 ms for the previous chunked-matmul kernel (serial carry chain +
fp32 DMA traffic was the bottleneck there).
"""

import math

import numpy as np

import concourse.bass as bass
import concourse.tile as tile
from concourse import mybir
from concourse.bass_utils import run_bass_kernel_spmd

EPS = 1e-4
N_CORES = 8
D = 512
NG = 4           # d-groups of 128 partitions
LB = 512         # tokens per scan link
SL = 8           # links per DMA slab

F16 = mybir.dt.float16
F32 = mybir.dt.float32

_prog_cache: dict = {}
last_results = None


def _legalize_waits(nc: bass.Bass) -> None:
    """walrus codegen allows one sync-wait per engine instruction; move any
    surplus waits onto injected same-engine no-ops right before it."""
    nid = 0
    for fn in nc.m.functions:
        for blk in fn.blocks:
            out = []
            changed = False
            for inst in blk.instructions:
                si = getattr(inst, "sync_info", None)
                waits = list(si.on_wait) if si is not None and si.on_wait else []
                if len(waits) > 1:
                    for w in waits[:-1]:
                        nop = mybir.InstNoOp(
                            name=f"waitnop-{nid}", text_hint="waitsplit"
                        )
                        nid += 1
                        nop.engine = inst.engine
                        nop.sync_info = mybir.SyncInfo(on_wait=[w], on_update=[])
                        out.append(nop)
                    inst.sync_info = mybir.SyncInfo(
                        on_wait=[waits[-1]], on_update=list(si.on_update)
                    )
                    changed = True
                out.append(inst)
            if changed:
                blk.instructions = out


def _slabs(nl: int, sl: int = SL):
    return [(l0, min(nl, l0 + sl)) for l0 in range(0, nl, sl)]


def _build_program(nl: int, loop_n: int = 0, lb: int = LB, sl: int = SL) -> bass.Bass:
    T = nl * lb
    nc = bass.Bass("TRN2", target_bir_lowering=False, debug=False,
                   num_devices=N_CORES)
    h_dev = nc.dram_tensor("h_dev", [128, NG * T], F16, kind="ExternalInput")
    a_dev = nc.dram_tensor("a_dev", [128, T], F16, kind="ExternalInput")
    out = nc.dram_tensor("out", [128, NG * T], F16, kind="ExternalOutput")

    import contextlib

    with tile.TileContext(nc) as tc:
        with (
            tc.tile_pool(name="hpool", bufs=2) as hpool,
            tc.tile_pool(name="apool", bufs=2) as apool,
            tc.tile_pool(name="ypool", bufs=2) as ypool,
            tc.tile_pool(name="bpool", bufs=4) as bpool,
        ):
            loop_ctx = tc.For_i(0, loop_n, 1) if loop_n else contextlib.nullcontext()
            with loop_ctx:
                prev = [None] * NG
                for (l0, l1) in _slabs(nl, sl):
                    ls = l1 - l0
                    TS = ls * lb
                    h_t = hpool.tile([128, NG * TS], F16, tag="h")
                    nc.sync.dma_start(h_t, h_dev[:, NG * l0 * lb : NG * l1 * lb])
                    a_t = apool.tile([128, TS], F16, tag="a")
                    nc.sync.dma_start(a_t, a_dev[:, l0 * lb : l1 * lb])
                    y_t = ypool.tile([128, NG * TS], F16, tag="y")
                    for li in range(ls):
                        a_blk = a_t[:, li * lb : (li + 1) * lb]
                        for g in range(NG):
                            o = (g * ls + li) * lb
                            b_t = bpool.tile([128, lb], F16, tag="b")
                            # b_t = (a-1)*h = -p*h  (scan tracks -y)
                            nc.vector.scalar_tensor_tensor(
                                b_t, a_blk, 1.0, h_t[:, o : o + lb],
                                mybir.AluOpType.subtract, mybir.AluOpType.mult,
                            )
                            init = 0.0 if prev[g] is None else prev[g]
                            nc.vector.tensor_tensor_scan(
                                y_t[:, o : o + lb], a_blk, b_t, init,
                                mybir.AluOpType.mult, mybir.AluOpType.add,
                            )
                            prev[g] = y_t[:, o + lb - 1 : o + lb]
                    nc.scalar.dma_start(
                        out[:, NG * l0 * lb : NG * l1 * lb], y_t
                    )
    _legalize_waits(nc)
    return nc


def _get_program(nl: int) -> bass.Bass:
    if nl not in _prog_cache:
        _prog_cache[nl] = _build_program(nl)
    return _prog_cache[nl]


def _split_ranges(starts: np.ndarray, length: int, k: int):
    """Partition [0,length) into k contiguous ranges cutting only at segment
    starts, minimizing the max range length. Returns list of (t0, t1)."""
    bounds = np.append(starts, length)
    lens = np.diff(bounds)
    nseg = len(lens)
    if nseg <= k:
        ranges = [(int(bounds[i]), int(bounds[i + 1])) for i in range(nseg)]
        ranges += [(length, length)] * (k - nseg)
        return ranges
    lo, hi = int(lens.max()), int(length)
    while lo < hi:
        mid = (lo + hi) // 2
        groups, cur = 1, 0
        for ln in lens:
            if cur + ln <= mid:
                cur += ln
            else:
                groups += 1
                cur = ln
        if groups <= k:
            hi = mid
        else:
            lo = mid + 1
    ranges = []
    s, cur = int(bounds[0]), 0
    for i, ln in enumerate(lens):
        if cur + ln > lo:
            ranges.append((s, int(bounds[i])))
            s, cur = int(bounds[i]), 0
        cur += int(ln)
    ranges.append((s, length))
    ranges += [(length, length)] * (k - len(ranges))
    return ranges


def _core_inputs(h_flat, a16, t0, t1, nl, lb: int = LB, sl: int = SL):
    T = nl * lb
    n = t1 - t0
    hp = np.zeros((T, D), np.float16)
    hp[:n] = h_flat[t0:t1].astype(np.float16)
    # hT4[dp, g, t] = h[t, g*128+dp]
    hT4 = np.ascontiguousarray(hp.T.reshape(NG, 128, T).transpose(1, 0, 2))
    ap = np.zeros(T, np.float16)
    ap[:n] = a16[t0:t1]
    h_cols = []
    for (l0, l1) in _slabs(nl, sl):
        h_cols.append(
            np.ascontiguousarray(hT4[:, :, l0 * lb : l1 * lb]).reshape(128, -1)
        )
    h_dev = np.concatenate(h_cols, axis=1)
    a_dev = np.ascontiguousarray(np.broadcast_to(ap[None, :], (128, T)))
    return h_dev, a_dev


def kernel(h_flat, b_flat, p_selected_flat, h_seq_idx):
    global last_results
    h_flat = np.ascontiguousarray(h_flat, np.float32)
    L, d = h_flat.shape
    assert d == D
    seg = np.asarray(h_seq_idx).reshape(-1).astype(np.int64)

    lo_f = np.float32(EPS)
    hi_f = np.float32(1.0 - EPS)
    p32 = np.clip(np.asarray(p_selected_flat, np.float32), lo_f, hi_f)
    a32 = 1.0 - p32  # decay exp(-dt) = exp(log1p(-p)) = 1-p exactly

    startf = np.empty(L, bool)
    startf[0] = True
    startf[1:] = seg[1:] != seg[:-1]
    a32 = np.where(startf, np.float32(0.0), a32)
    a16 = a32.astype(np.float16)
    # At segment starts a is forced to 0, so the device's gate (1-a)=1 is
    # wrong there; fold the true gate p into h for those few rows.
    h_flat = h_flat.copy()
    h_flat[startf] *= p32[startf, None]

    idx = np.cumsum(np.asarray(b_flat, np.int64)) - 1

    ranges = _split_ranges(np.flatnonzero(startf), L, N_CORES)
    maxlen = max(t1 - t0 for t0, t1 in ranges)
    nl = max(math.ceil(maxlen / LB), 1)

    nc = _get_program(nl)
    T = nl * LB

    in_maps = []
    for t0, t1 in ranges:
        h_dev, a_dev = _core_inputs(h_flat, a16, t0, t1, nl)
        in_maps.append({"h_dev": h_dev, "a_dev": a_dev})

    res = run_bass_kernel_spmd(
        nc, in_maps, core_ids=list(range(N_CORES)), trace=False
    )
    last_results = res

    y = np.empty((L, D), np.float32)
    for i, (t0, t1) in enumerate(ranges):
        n = t1 - t0
        if not n:
            continue
        ydev = res.results[i]["out"]
        yT4 = np.empty((128, NG, T), np.float16)
        c0 = 0
        for (l0, l1) in _slabs(nl):
            TS = (l1 - l0) * LB
            yT4[:, :, l0 * LB : l1 * LB] = ydev[:, c0 : c0 + NG * TS].reshape(
                128, NG, TS
            )
            c0 += NG * TS
        # yT4[dp, g, t] = -y[t, g*128+dp]
        yt = yT4.transpose(2, 1, 0).reshape(T, D)[:n]
        y[t0:t1] = -yt.astype(np.float32)

    gidx = np.where(idx < 0, idx + L, idx)
    gidx = np.clip(gidx, 0, L - 1)
    return y[gidx]
